# revision 3
# baseline (speedup 1.0000x reference)
"""Bionetwork sparse-matvec recurrence on 8 trn2 NeuronCores.

y_{t+1} = act(A y_t + b_in), 150 iterations, A fixed sparse (3.2M edges,
100k nodes).  Dest-sharded across 8 cores; all routing tables SBUF-resident.

Per iteration, per core (local_scatter = vectorized GPSIMD within-row scatter):
  1. seed-scatter per dest-chunk g: canonical y -> run-starts of expansion
  2. log-fill (DVE, masked shifted adds) completes source runs (len<=8)
  3. multiply by edge weights (fp16, in place)
  4. round-1 local_scatter: products -> staging tiles at col 128*t + dest_row
  5. PE transpose of each [128,128] staging tile (the cross-partition hop)
  6. round-2 local_scatter: transposed stream -> dest-slot layout
  7. segmented reduce (32-wide slots) -> fp32; fold pseudo-slot regions
  8. v = s + b_in; piecewise activation; write shard; AllGather; reload y

Everything is table-driven; tables are built host-side from the (fixed)
edge lists and shipped as per-core input tensors to one shared program.
"""
import numpy as np

N = 100000
E = 3200000
P = 128
NCORES = 8
QW = 800                    # canonical width: 128*800 = 102400
NC_PAD = P * QW
SHARD = NC_PAD // NCORES    # 12800 = 128*100
KMAX = SHARD // P           # 100
ITERS = 150
LEAK = 0.01
RUN_CAP = 16                # fill rounds 1,2,4,8 cover runs of 16
SEED_REGIONS = 1
MAX_DST = 2046
TILES_PER_CALL = 15
SD = SEED_REGIONS * QW


def _ceil(a, b):
    return -(-a // b)


def _prep(x, in_weights, rec_weights, biases, out_weights,
          in_indices, edge_rows, edge_cols, out_indices):
    deg = np.bincount(edge_rows, minlength=N)
    npseudo = np.maximum(1, _ceil(deg, 32))
    assert npseudo.max() <= 4, f"max in-degree {deg.max()} > 128 unsupported"

    # deal dests round-robin over 1024 (core,row) bins; sort by npseudo desc
    # (region contiguity) but shuffle within classes (chunk load balance)
    rng = np.random.default_rng(12345)
    order = np.lexsort((rng.permutation(N), -npseudo))
    i = np.arange(N)
    b = i % (NCORES * P)
    core_of, row_of, k_of = b % NCORES, b // NCORES, i // (NCORES * P)
    Kreal = int(k_of.max()) + 1
    assert Kreal <= KMAX
    perm = np.empty(N, np.int64)
    perm[order] = SHARD * core_of + KMAX * row_of + k_of

    nr_max = {r: _ceil(int((npseudo >= r).sum()), NCORES * P) for r in (2, 3, 4)}
    region_base = {1: 0}
    base = Kreal
    for r in (2, 3, 4):
        region_base[r] = base
        base += nr_max[r]
    KP = base
    FD = 32 * KP
    NCH = _ceil(FD, MAX_DST)
    CH = _ceil(_ceil(FD, NCH), 32) * 32
    NCH = _ceil(FD, CH)

    import jax.numpy as jnp
    node_in = np.asarray(
        jnp.zeros((N,), jnp.float32).at[jnp.asarray(in_indices)].set(
            jnp.asarray(in_weights, jnp.float32) * jnp.asarray(x[0], jnp.float32)))
    b_in_full = node_in + biases.astype(np.float32)

    dnew, snew = perm[edge_rows], perm[edge_cols]
    w_all = rec_weights.astype(np.float32)
    dcore = dnew // SHARD

    # ---------- pass 1: per-core edge geometry ----------
    geo = []
    for c in range(NCORES):
        em = np.where(dcore == c)[0]
        d_loc = dnew[em] - SHARD * c
        j, k = d_loc // KMAX, d_loc % KMAX
        s_new = snew[em]
        p0, q0 = s_new // QW, s_new % QW
        w = w_all[em]
        ne = em.size

        def ranks_of(key):
            so = np.argsort(key, kind="stable")
            ks = key[so]
            st = np.r_[0, np.flatnonzero(np.diff(ks)) + 1]
            sid = np.zeros(ne, np.int64)
            sid[st[1:]] = 1
            sid = np.cumsum(sid)
            r = np.arange(ne) - st[sid]
            out = np.empty(ne, np.int64)
            out[so] = r
            return out

        slot = ranks_of(d_loc)
        r_idx = slot // 32
        rbv = np.array([region_base[1], region_base[2], region_base[3], region_base[4]])
        f = 32 * (rbv[r_idx] + k) + slot % 32
        g = f // CH
        trank = ranks_of((g * P + p0) * P + j)
        # expansion position within (g,p0) ordered by q0, and rank within source
        so3 = np.lexsort((q0, p0, g))
        gp = (g * P + p0)[so3]
        st = np.r_[0, np.flatnonzero(np.diff(gp)) + 1]
        sid = np.zeros(ne, np.int64)
        sid[st[1:]] = 1
        sid = np.cumsum(sid)
        m_pos = np.empty(ne, np.int64)
        m_pos[so3] = np.arange(ne) - st[sid]
        gpq = ((g * P + p0) * QW + q0)[so3]
        st4 = np.r_[0, np.flatnonzero(np.diff(gpq)) + 1]
        sid4 = np.zeros(ne, np.int64)
        sid4[st4[1:]] = 1
        sid4 = np.cumsum(sid4)
        src_rank = np.empty(ne, np.int64)
        src_rank[so3] = np.arange(ne) - st4[sid4]
        assert int(src_rank.max()) < RUN_CAP * SEED_REGIONS
        geo.append(dict(j=j, p0=p0, q0=q0, w=w, f=f, g=g,
                        trank=trank, m_pos=m_pos, src_rank=src_rank, ne=ne))

    # uniform per-chunk sizes across cores
    M1 = np.zeros(NCH, np.int64)
    MTg = np.zeros(NCH, np.int64)
    for gg in geo:
        for g2 in range(NCH):
            sel = gg["g"] == g2
            if sel.any():
                M1[g2] = max(M1[g2], int(gg["m_pos"][sel].max()) + 1)
                MTg[g2] = max(MTg[g2], int(gg["trank"][sel].max()) + 1)
    M1 = (_ceil(M1, 2) * 2).astype(np.int64)
    EB = np.r_[0, np.cumsum(M1)]         # expansion bases
    MEXP = int(EB[-1])
    TBASE = np.r_[0, np.cumsum(MTg)]     # tile bases
    T = int(TBASE[-1])
    # round-1 call structure: (g, t0, t1) uniform
    r1_struct = []
    for g2 in range(NCH):
        for t0 in range(0, int(MTg[g2]), TILES_PER_CALL):
            r1_struct.append((g2, t0, min(t0 + TILES_PER_CALL, int(MTg[g2]))))
    NR1 = len(r1_struct)

    # ---------- pass 2: tables ----------
    cores = []
    for c in range(NCORES):
        gg = geo[c]
        j, p0, q0, w = gg["j"], gg["p0"], gg["q0"], gg["w"]
        f, g, trank, m_pos, src_rank = (gg["f"], gg["g"], gg["trank"],
                                        gg["m_pos"], gg["src_rank"])
        m_glob = EB[g] + m_pos
        dist = src_rank

        seedidx = np.full((NCH, P, SD), -1, np.int16)
        sm = dist == 0
        seedidx[g[sm], p0[sm], q0[sm]] = m_pos[sm].astype(np.int16)

        masks = np.zeros((4, P, MEXP), np.float16)
        for ki, kk in enumerate((1, 2, 4, 8)):
            mm = dist >= kk
            masks[ki, p0[mm], m_glob[mm]] = 1.0

        w_exp = np.zeros((P, MEXP), np.float16)
        w_exp[p0, m_glob] = w.astype(np.float16)

        idx1 = []
        for (g2, t0, t1) in r1_struct:
            sel = (g == g2) & (trank >= t0) & (trank < t1)
            idx = np.full((P, int(M1[g2])), -1, np.int16)
            idx[p0[sel], m_pos[sel]] = (128 * (trank[sel] - t0) + j[sel]).astype(np.int16)
            idx1.append(idx)

        idx2 = []
        for g2 in range(NCH):
            sel = g == g2
            idx = np.full((P, 128 * int(MTg[g2])), -1, np.int16)
            idx[j[sel], 128 * trank[sel] + p0[sel]] = (f[sel] - g2 * CH).astype(np.int16)
            idx2.append(idx)

        b_in_t = np.zeros((P, Kreal), np.float32)
        nid = np.where((perm >= SHARD * c) & (perm < SHARD * (c + 1)))[0]
        dl = perm[nid] - SHARD * c
        b_in_t[dl // KMAX, dl % KMAX] = b_in_full[nid]

        cores.append(dict(seedidx=seedidx, masks=masks, w_exp=w_exp,
                          idx1=idx1, idx2=idx2, b_in_t=b_in_t))

    meta = dict(Kreal=Kreal, KP=KP, FD=FD, NCH=NCH, CH=CH, M1=M1, EB=EB,
                MTg=MTg, TBASE=TBASE, T=T, MEXP=MEXP, NR1=NR1,
                r1_struct=r1_struct, nr_max=nr_max, region_base=region_base)
    return cores, perm, meta


def _act_np(v):
    y1 = np.maximum(v, np.float32(LEAK) * v)
    ysat = (1.0 - 0.25 / np.maximum(v, 0.5)).astype(v.dtype)
    return np.where(v > 0.5, ysat, y1)


def _sim(cores, perm, meta, n_iters, quant=True):
    dt = np.float16 if quant else np.float32
    Kreal, KP, FD, NCH, CH = (meta["Kreal"], meta["KP"], meta["FD"],
                              meta["NCH"], meta["CH"])
    M1, EB, MTg, TBASE, T, MEXP = (meta["M1"], meta["EB"], meta["MTg"],
                                   meta["TBASE"], meta["T"], meta["MEXP"])
    y = np.zeros(NC_PAD, np.float32)
    for it in range(n_iters):
        y2d = y.reshape(P, QW).astype(dt)
        seed_data = y2d
        y_next = np.zeros(NC_PAD, np.float32)
        for c, tb in enumerate(cores):
            exp_t = np.zeros((P, MEXP), dt)
            for g2 in range(NCH):
                sidx = tb["seedidx"][g2]
                pp, cc = np.where(sidx >= 0)
                exp_t[pp, EB[g2] + sidx[pp, cc]] = seed_data[pp, cc]
            for ki, kk in enumerate((1, 2, 4, 8)):
                sh = np.zeros_like(exp_t)
                sh[:, kk:] = exp_t[:, :-kk]
                exp_t = (exp_t + tb["masks"][ki].astype(dt) * sh).astype(dt)
            prod = (exp_t.astype(np.float32) * tb["w_exp"].astype(np.float32)).astype(dt)
            staging = np.zeros((P, 128 * T), dt)
            for ci, (g2, t0, t1) in enumerate(meta["r1_struct"]):
                idx = tb["idx1"][ci]
                data = prod[:, EB[g2]:EB[g2] + M1[g2]]
                pp, cc = np.where(idx >= 0)
                staging[pp, 128 * (TBASE[g2] + t0) + idx[pp, cc]] = data[pp, cc]
            t2 = np.zeros_like(staging)
            for t in range(T):
                t2[:, 128 * t:128 * (t + 1)] = staging[:, 128 * t:128 * (t + 1)].T
            slots = np.zeros((P, FD), dt)
            for g2 in range(NCH):
                idx = tb["idx2"][g2]
                data = t2[:, 128 * TBASE[g2]:128 * (TBASE[g2] + MTg[g2])]
                pp, cc = np.where(idx >= 0)
                slots[pp, g2 * CH + idx[pp, cc]] = data[pp, cc]
            sp = slots.reshape(P, KP, 32).astype(np.float32).sum(axis=2)
            s = sp[:, :Kreal].copy()
            for r in (2, 3, 4):
                nr = meta["nr_max"][r]
                if nr:
                    b0 = meta["region_base"][r]
                    s[:, :nr] += sp[:, b0:b0 + nr]
            v = s + tb["b_in_t"]
            y32 = _act_np(v)
            jj, kk2 = np.meshgrid(np.arange(P), np.arange(Kreal), indexing="ij")
            y_next[SHARD * c + KMAX * jj.ravel() + kk2.ravel()] = y32.ravel()
        y = y_next
    return y


# ============================ BASS KERNEL ============================

def _build(cores, meta, n_iters, no_cc=False):
    import concourse.bacc as bacc
    import concourse.mybir as mybir
    import concourse.tile as tile
    from concourse.masks import make_identity

    f16, f32, i16 = mybir.dt.float16, mybir.dt.float32, mybir.dt.int16
    AOP = mybir.AluOpType
    Kreal, KP, FD, NCH, CH = (meta["Kreal"], meta["KP"], meta["FD"],
                              meta["NCH"], meta["CH"])
    M1, EB, MTg, TBASE, T, MEXP, NR1 = (meta["M1"], meta["EB"], meta["MTg"],
                                        meta["TBASE"], meta["T"],
                                        meta["MEXP"], meta["NR1"])
    DSTW = [min(FD, (g + 1) * CH) - g * CH for g in range(NCH)]

    nc = bacc.Bacc("TRN2", target_bir_lowering=False)

    d_seed = [nc.dram_tensor(f"t_seed{g}", [P, SD], i16, kind="ExternalInput")
              for g in range(NCH)]
    d_mask = [nc.dram_tensor(f"t_mask{k}", [P, MEXP], f16, kind="ExternalInput")
              for k in range(4)]
    d_wexp = nc.dram_tensor("t_wexp", [P, MEXP], f16, kind="ExternalInput")
    d_idx1 = [nc.dram_tensor(f"t_idx1_{ci}", [P, int(M1[g2])], i16,
                             kind="ExternalInput")
              for ci, (g2, _, _) in enumerate(meta["r1_struct"])]
    d_idx2 = [nc.dram_tensor(f"t_idx2_{g}", [P, 128 * int(MTg[g])], i16,
                             kind="ExternalInput") for g in range(NCH)]
    d_bin = nc.dram_tensor("t_bin", [P, Kreal], f32, kind="ExternalInput")
    d_yout = nc.dram_tensor("y_out", [P, Kreal], f32, kind="ExternalOutput")
    d_ysh = nc.dram_tensor("y_shard", [1, SHARD], f16, kind="Internal")
    # [P, QW] view of the same row-major buffer: the cost model prices the
    # collective by the out-AP's per-partition free bytes.
    d_yfull = nc.dram_tensor("y_full", [P, QW], f16, kind="Internal",
                             addr_space="Shared")
    d_yin = nc.dram_tensor("y_in", [1, NC_PAD], f16, kind="ExternalInput")
    d_yall = nc.dram_tensor("y_all", [1, NC_PAD], f16, kind="ExternalOutput")

    with tile.TileContext(nc) as tc:
        with tc.tile_pool(name="tables", bufs=1) as tp, \
             tc.tile_pool(name="psum", bufs=8, space="PSUM") as pp:
            t_seed = [tp.tile([P, SD], i16, name=f"seed{g}") for g in range(NCH)]
            t_mask = [tp.tile([P, MEXP], f16, name=f"mask{k}") for k in range(4)]
            t_wexp = tp.tile([P, MEXP], f16, name="wexp")
            t_idx1 = [tp.tile([P, int(M1[g2])], i16, name=f"i1_{ci}")
                      for ci, (g2, _, _) in enumerate(meta["r1_struct"])]
            t_idx2 = [tp.tile([P, 128 * int(MTg[g])], i16, name=f"i2_{g}")
                      for g in range(NCH)]
            t_bin = tp.tile([P, Kreal], f32, name="bin")
            ident = tp.tile([P, P], f16, name="ident")
            y2d = tp.tile([P, QW], f16, name="y2d")
            expb = [tp.tile([P, int(M1[g])], f16, name=f"expb{g}")
                    for g in range(NCH)]
            tmpb = [tp.tile([P, int(M1[g])], f16, name=f"tmpb{g}")
                    for g in range(NCH)]
            stag = [tp.tile([P, 128 * int(MTg[g])], f16, name=f"stag{g}")
                    for g in range(NCH)]
            t2d = [tp.tile([P, 128 * int(MTg[g])], f16, name=f"t2d{g}")
                   for g in range(NCH)]
            slots = [tp.tile([P, DSTW[g] // 32, 32], f16, name=f"slots{g}")
                     for g in range(NCH)]
            sp = tp.tile([P, KP], f32, name="sp")
            vv = tp.tile([P, Kreal], f32, name="vv")
            y1b = tp.tile([P, Kreal], f32, name="y1b")
            rb = tp.tile([P, Kreal], f32, name="rb")
            mb = tp.tile([P, Kreal], f32, name="mb")
            y32 = tp.tile([P, Kreal], f32, name="y32")
            y16 = tp.tile([P, KMAX], f16, name="y16")

            for g in range(NCH):
                nc.sync.dma_start(t_seed[g][:], d_seed[g][:])
                nc.sync.dma_start(t_idx2[g][:], d_idx2[g][:])
            for k in range(4):
                nc.sync.dma_start(t_mask[k][:], d_mask[k][:])
            for ci in range(NR1):
                nc.sync.dma_start(t_idx1[ci][:], d_idx1[ci][:])
            nc.sync.dma_start(t_wexp[:], d_wexp[:])
            nc.sync.dma_start(t_bin[:], d_bin[:])
            make_identity(nc, ident[:])
            nc.sync.dma_start(y2d[:], d_yin[:].rearrange("o (p q) -> (o p) q", p=P))
            nc.vector.memset(y16[:], 0.0)

            r1_by_g = {}
            for ci, (g2, t0, t1) in enumerate(meta["r1_struct"]):
                r1_by_g.setdefault(g2, []).append((ci, t0, t1))

            def body(iv=None):
                for g in range(NCH):
                    w0, w1 = int(EB[g]), int(EB[g + 1])
                    mw = int(M1[g])
                    # expansion for chunk g
                    nc.gpsimd.local_scatter(
                        expb[g][:], y2d[:], t_seed[g][:],
                        channels=P, num_elems=mw, num_idxs=SD)
                    for ki, kk in enumerate((1, 2, 4, 8)):
                        nc.vector.memset(tmpb[g][:, 0:kk], 0.0)
                        nc.vector.tensor_tensor(
                            tmpb[g][:, kk:mw], expb[g][:, 0:mw - kk],
                            t_mask[ki][:, w0 + kk:w1], op=AOP.mult)
                        nc.vector.tensor_tensor(expb[g][:], expb[g][:],
                                                tmpb[g][:], op=AOP.add)
                    nc.vector.tensor_tensor(expb[g][:], expb[g][:],
                                            t_wexp[:, w0:w1], op=AOP.mult)
                    # round 1 into per-chunk staging
                    for ci, t0, t1 in r1_by_g[g]:
                        nt = t1 - t0
                        nc.gpsimd.local_scatter(
                            stag[g][:, 128 * t0:128 * t1], expb[g][:],
                            t_idx1[ci][:], channels=P, num_elems=128 * nt,
                            num_idxs=mw)
                    # transposes
                    Tg = int(MTg[g])
                    for tb0 in range(0, Tg, 8):
                        nb = min(8, Tg - tb0)
                        pt = pp.tile([P, 8 * P], f16, space="PSUM", tag="tr",
                                     name="tr")
                        for t in range(tb0, tb0 + nb):
                            nc.tensor.transpose(
                                pt[:, 128 * (t - tb0):128 * (t - tb0 + 1)],
                                stag[g][:, 128 * t:128 * (t + 1)], ident[:])
                        nc.scalar.copy(
                            t2d[g][:, 128 * tb0:128 * (tb0 + nb)],
                            pt[:, 0:128 * nb])
                    # round 2 into dest slots
                    nc.gpsimd.local_scatter(
                        slots[g][:].rearrange("p k s -> p (k s)"), t2d[g][:],
                        t_idx2[g][:], channels=P, num_elems=DSTW[g],
                        num_idxs=128 * Tg)
                    # segmented reduce for chunk g
                    c0 = g * CH // 32
                    nc.vector.tensor_reduce(
                        sp[:, c0:c0 + DSTW[g] // 32], slots[g][:],
                        axis=mybir.AxisListType.X, op=AOP.add)
                for r in (2, 3, 4):
                    nr = meta["nr_max"][r]
                    if nr:
                        b0 = meta["region_base"][r]
                        nc.vector.tensor_tensor(sp[:, 0:nr], sp[:, 0:nr],
                                                sp[:, b0:b0 + nr], op=AOP.add)
                nc.vector.tensor_tensor(vv[:], sp[:, 0:Kreal], t_bin[:], op=AOP.add)
                nc.vector.scalar_tensor_tensor(
                    y1b[:], vv[:], float(LEAK), vv[:], op0=AOP.mult, op1=AOP.max)
                nc.vector.tensor_scalar_max(rb[:], vv[:], 0.5)
                nc.vector.reciprocal(rb[:], rb[:])
                nc.vector.tensor_scalar(rb[:], rb[:], -0.25, 1.0,
                                        op0=AOP.mult, op1=AOP.add)
                nc.vector.tensor_scalar(mb[:], vv[:], 0.5, None, op0=AOP.is_gt)
                nc.vector.tensor_tensor(rb[:], rb[:], y1b[:], op=AOP.subtract)
                nc.vector.tensor_tensor(mb[:], mb[:], rb[:], op=AOP.mult)
                nc.vector.tensor_tensor(y32[:], y1b[:], mb[:], op=AOP.add)
                nc.vector.tensor_copy(y16[:, 0:Kreal], y32[:])
                nc.sync.dma_start(
                    d_ysh[:].rearrange("o (p k) -> (o p) k", p=P), y16[:])
                if not no_cc:
                    nc.gpsimd.collective_compute(
                        "AllGather", AOP.bypass,
                        replica_groups=[list(range(NCORES))],
                        ins=[d_ysh[:]], outs=[d_yfull[:]])
                nc.sync.dma_start(y2d[:], d_yfull[:])

            for _ in range(n_iters):
                body()
            nc.sync.dma_start(d_yout[:], y32[:])
            nc.sync.dma_start(
                d_yall[:].rearrange("o (p q) -> (o p) q", p=P), y2d[:])

    nc.compile()
    return nc


def _in_maps(cores, meta):
    maps = []
    for tb in cores:
        m = {"t_wexp": tb["w_exp"], "t_bin": tb["b_in_t"]}
        for g in range(meta["NCH"]):
            m[f"t_seed{g}"] = tb["seedidx"][g]
            m[f"t_idx2_{g}"] = tb["idx2"][g]
        for k in range(4):
            m[f"t_mask{k}"] = np.ascontiguousarray(tb["masks"][k])
        for ci in range(meta["NR1"]):
            m[f"t_idx1_{ci}"] = tb["idx1"][ci]
        maps.append(m)
    return maps


def _gather_y(res, meta):
    Kreal = meta["Kreal"]
    y_full = np.zeros(NC_PAD, np.float32)
    jj, kk2 = np.meshgrid(np.arange(P), np.arange(Kreal), indexing="ij")
    for c in range(NCORES):
        y32 = res.results[c]["y_out"]
        y_full[SHARD * c + KMAX * jj.ravel() + kk2.ravel()] = y32.ravel()
    return y_full


SEG = 150  # whole run fits one NEFF


def kernel(**inputs):
    from concourse.bass_utils import run_bass_kernel_spmd
    inputs = {k: np.asarray(v) for k, v in inputs.items()}
    cores, perm, meta = _prep(**inputs)
    nseg = _ceil(ITERS, SEG)
    nc = _build(cores, meta, SEG)
    maps = _in_maps(cores, meta)
    y_state = np.zeros((1, NC_PAD), np.float16)
    res = None
    for s in range(nseg):
        for m in maps:
            m["y_in"] = y_state
        res = run_bass_kernel_spmd(nc, [dict(m) for m in maps],
                                   core_ids=list(range(NCORES)))
        y_state = res.results[0]["y_all"]
    y_old = _gather_y(res, meta)[perm]
    out = (inputs["out_weights"].astype(np.float32)
           * y_old[inputs["out_indices"]])[None, :]
    return out.astype(np.float32)


if __name__ == "__main__":
    import sys, time
    sys.path.insert(0, "/root/problem")
    import reference
    inputs = {k: np.asarray(v) for k, v in reference.setup_inputs().items()}
    t0 = time.time()
    cores, perm, meta = _prep(**inputs)
    print(f"prep {time.time()-t0:.1f}s Kreal={meta['Kreal']} KP={meta['KP']} "
          f"FD={meta['FD']} M1={meta['M1']} MTg={meta['MTg']} T={meta['T']} "
          f"MEXP={meta['MEXP']} NR1={meta['NR1']}")
    if "sim" in sys.argv:
        n_it = int(sys.argv[sys.argv.index("sim") + 1]) if len(sys.argv) > 2 else 8
        import jax.numpy as jnp
        ni = np.asarray(jnp.zeros((N,), jnp.float32).at[jnp.asarray(inputs["in_indices"])].set(
            jnp.asarray(inputs["in_weights"], jnp.float32) * jnp.asarray(inputs["x"][0], jnp.float32)))
        b_in = (ni + inputs["biases"]).astype(np.float64)
        rw = inputs["rec_weights"].astype(np.float64)
        er, ec = inputs["edge_rows"], inputs["edge_cols"]
        yref = np.zeros(N, np.float64)
        for _ in range(n_it):
            s = np.bincount(er, weights=rw * yref[ec], minlength=N)
            v = s + b_in
            yref = np.where(v > 0.5, 1.0 - 0.25 / np.maximum(v, 0.5),
                            np.maximum(v, LEAK * v))
        scale = np.abs(yref).max()
        t0 = time.time()
        ys = _sim(cores, perm, meta, n_it, quant=False)
        print(f"sim(noquant,{n_it}) {time.time()-t0:.1f}s  max rel err:",
              np.abs(ys[perm] - yref).max() / scale)
        t0 = time.time()
        ysq = _sim(cores, perm, meta, n_it, quant=True)
        print(f"sim(fp16,{n_it}) {time.time()-t0:.1f}s  max rel err:",
              np.abs(ysq[perm] - yref).max() / scale)



# revision 6
# speedup vs baseline: 1.2850x; 1.2850x over previous
"""Bionetwork sparse-matvec recurrence on 8 trn2 NeuronCores — v2.

y_{t+1} = act(A y_t + b_in), 150 iterations, A fixed sparse (3.2M edges,
100k nodes).  Dest-sharded across 8 cores; all routing tables SBUF-resident.

v2 changes vs v1:
  - chunks balanced by edge count AND tile-cell maxima (greedy + swaps),
    cutting GPSIMD staging scans
  - overflow tiles (trank >= 15) packed into an expansion suffix so the
    second round-1 scatter call scans a narrow table
  - class-2 destination columns get interleaved 64-wide slots (single
    segmented reduce, no fold passes)
  - pipeline-ordered emission so Pool/DVE/PE/Act overlap across chunks
"""
import numpy as np

N = 100000
E = 3200000
P = 128
NCORES = 8
K16 = 98                    # y16 columns per core
QW = NCORES * K16           # 784  canonical y2d width
NC_PAD = P * QW             # 100352
SHARD = P * K16             # 12544
NCH = 3
ITERS = 150
LEAK = 0.01
RUN_CAP = 8                 # fill rounds 1,2,4; seeds every 8
SEED_REG = 2                # seed regions (src_rank 0 and 8)
TILES_PER_CALL = 15         # 128*15 = 1920 <= 2046 local_scatter cap
MAX_SLOT_ELEMS = 2046


def _ceil(a, b):
    return -(-a // b)


def _ranks_of(key):
    """rank of each element within its equal-key group (stable)."""
    ne = key.shape[0]
    so = np.argsort(key, kind="stable")
    ks = key[so]
    st = np.r_[0, np.flatnonzero(np.diff(ks)) + 1]
    sid = np.zeros(ne, np.int64)
    sid[st[1:]] = 1
    sid = np.cumsum(sid)
    r = np.arange(ne) - st[sid]
    out = np.empty(ne, np.int64)
    out[so] = r
    return out


def _grp_pos(key, subsort):
    """m_pos: rank within key-group under (key, subsort) stable order, and
    src_rank: rank within (key, subsort)-group."""
    ne = key.shape[0]
    so = np.lexsort((subsort, key))
    kso = key[so]
    st = np.r_[0, np.flatnonzero(np.diff(kso)) + 1]
    sid = np.zeros(ne, np.int64)
    sid[st[1:]] = 1
    sid = np.cumsum(sid)
    mp = np.empty(ne, np.int64)
    mp[so] = np.arange(ne) - st[sid]
    k2 = kso * (subsort.max() + 1) + subsort[so]
    st2 = np.r_[0, np.flatnonzero(np.diff(k2)) + 1]
    sid2 = np.zeros(ne, np.int64)
    sid2[st2[1:]] = 1
    sid2 = np.cumsum(sid2)
    sr = np.empty(ne, np.int64)
    sr[so] = np.arange(ne) - st2[sid2]
    return mp, sr


def _prep(x, in_weights, rec_weights, biases, out_weights,
          in_indices, edge_rows, edge_cols, out_indices):
    deg = np.bincount(edge_rows, minlength=N)
    npseudo = np.maximum(1, _ceil(deg, 32))
    assert npseudo.max() <= 2, f"max in-degree {deg.max()} > 64 unsupported"

    # deal dests round-robin over 1024 (core,row) bins, heavy nodes first so
    # pseudo-slot classes are (nearly) a function of the column index
    rng = np.random.default_rng(12345)
    order = np.lexsort((rng.permutation(N), -npseudo))
    i = np.arange(N)
    b = i % (NCORES * P)
    node_core = np.empty(N, np.int64)
    node_row = np.empty(N, np.int64)
    node_kk = np.empty(N, np.int64)
    node_core[order] = b % NCORES
    node_row[order] = b // NCORES
    node_kk[order] = i // (NCORES * P)
    Kreal = int(node_kk.max()) + 1
    assert Kreal == K16

    col_np = np.zeros(Kreal, np.int64)
    np.maximum.at(col_np, node_kk, npseudo)

    # ---------- greedy column->chunk packing ----------
    flat0 = SHARD * node_core + K16 * node_row + node_kk
    dc = node_core[edge_rows]
    dj = node_row[edge_rows]
    dkk = node_kk[edge_rows]
    sp0 = flat0[edge_cols] // QW
    cellidx = ((dkk * NCORES + dc) * P + dj) * P + sp0
    colcells = np.bincount(cellidx, minlength=Kreal * NCORES * P * P)
    colcells = colcells.reshape(Kreal, NCORES, P, P).astype(np.int32)
    coledges = colcells.reshape(Kreal, -1).sum(axis=1)

    chunk_cells = np.zeros((NCH, NCORES, P, P), np.int32)
    chunk_edges = np.zeros(NCH, np.int64)
    chunk_cols = [[] for _ in range(NCH)]
    target_cols = _ceil(Kreal, NCH)
    for k in np.argsort(-coledges):
        best, bkey = None, None
        for g in range(NCH):
            if len(chunk_cols[g]) >= target_cols:
                continue
            cand = chunk_cells[g] + colcells[k]
            mx = int(cand.max())
            over = int((cand >= mx - 1).sum())
            key = (mx, over, chunk_edges[g])
            if bkey is None or key < bkey:
                best, bkey = g, key
        g = best
        chunk_cells[g] += colcells[k]
        chunk_edges[g] += coledges[k]
        chunk_cols[g].append(int(k))

    # local search: same-class column swaps minimizing sum of per-chunk tile
    # maxima (= Pool staging scans), tie-broken by the sorted maxima
    def _key(maxima):
        return (sum(maxima), sorted(maxima, reverse=True))
    cur = _key([int(chunk_cells[g].max()) for g in range(NCH)])
    rng2 = np.random.default_rng(999)
    stall = 0
    for it in range(3000):
        if stall > 60:
            break
        maxima = [int(chunk_cells[g].max()) for g in range(NCH)]
        gmax = int(np.argmax(maxima)) if it % 2 == 0 else int(rng2.integers(NCH))
        hot = np.unravel_index(np.argmax(chunk_cells[gmax]),
                               chunk_cells[gmax].shape)
        cols_h = chunk_cols[gmax]
        contrib = np.array([colcells[k][hot] for k in cols_h])
        ka = cols_h[int(rng2.choice(np.argsort(-contrib)[:3]))]
        improved = False
        for gb in rng2.permutation(NCH):
            if gb == gmax:
                continue
            for kb in rng2.permutation(chunk_cols[gb])[:20]:
                kb = int(kb)
                if col_np[ka] != col_np[kb]:
                    continue
                na = chunk_cells[gmax] - colcells[ka] + colcells[kb]
                nb = chunk_cells[gb] - colcells[kb] + colcells[ka]
                ms = [int(na.max()) if g == gmax else
                      int(nb.max()) if g == gb else
                      int(chunk_cells[g].max()) for g in range(NCH)]
                ns = _key(ms)
                if ns < cur:
                    chunk_cells[gmax] = na
                    chunk_cells[gb] = nb
                    chunk_cols[gmax][chunk_cols[gmax].index(ka)] = kb
                    chunk_cols[gb][chunk_cols[gb].index(kb)] = int(ka)
                    ed = coledges[ka] - coledges[kb]
                    chunk_edges[gmax] -= ed
                    chunk_edges[gb] += ed
                    cur = ns
                    improved = True
                    break
            if improved:
                break
        stall = 0 if improved else stall + 1

    # final column order: per chunk, class-2 first then class-1 (kk asc)
    kmap = np.empty(Kreal, np.int64)
    K_g, NR2_g = [], []
    pos = 0
    for g in range(NCH):
        cols = sorted(chunk_cols[g], key=lambda k: (-col_np[k], k))
        for k in cols:
            kmap[k] = pos
            pos += 1
        K_g.append(len(cols))
        NR2_g.append(int(sum(1 for k in cols if col_np[k] >= 2)))
    assert pos == Kreal
    node_k = kmap[node_kk]

    # chunk geometry: per chunk, slot layout [class2 cols: 64-wide][class1: 32]
    KB_g = np.r_[0, np.cumsum(K_g)]
    KP_g = [K_g[g] + NR2_g[g] for g in range(NCH)]       # 32-slot columns
    assert all(32 * kp <= MAX_SLOT_ELEMS for kp in KP_g), KP_g
    KPB_g = np.r_[0, np.cumsum(KP_g)]
    KP = int(KPB_g[-1])
    chunk_of_k = np.empty(Kreal, np.int64)
    cp_of_k = np.empty(Kreal, np.int64)
    for g in range(NCH):
        sel = np.arange(KB_g[g], KB_g[g + 1])
        chunk_of_k[sel] = g
        cp_of_k[sel] = sel - KB_g[g]

    # final canonical map
    flat = SHARD * node_core + K16 * node_row + node_k
    perm = flat
    p0_of = flat // QW
    q0_of = flat % QW

    import jax.numpy as jnp
    node_in = np.asarray(
        jnp.zeros((N,), jnp.float32).at[jnp.asarray(in_indices)].set(
            jnp.asarray(in_weights, jnp.float32) * jnp.asarray(x[0], jnp.float32)))
    b_in_full = node_in + biases.astype(np.float32)

    # ---------- per-edge geometry ----------
    dk = node_k[edge_rows]
    g_of = chunk_of_k[dk]
    sp0 = p0_of[edge_cols]
    sq0 = q0_of[edge_cols]
    w_all = rec_weights.astype(np.float32)

    # slot position: class-2 col c -> 64 slots at 64*cp; class-1 col ->
    # 32 slots at 64*NR2 + 32*(cp - NR2)
    destid = dc * (P * K16) + dj * K16 + dk
    drank = _ranks_of(destid)
    nr2 = np.array([NR2_g[g] for g in range(NCH)])
    cp = cp_of_k[dk]
    is2 = cp < nr2[g_of]
    off_in_chunk = np.where(is2, 64 * cp + drank,
                            64 * nr2[g_of] + 32 * (cp - nr2[g_of]) + drank)
    assert int((drank >= np.where(is2, 64, 32)).sum()) == 0
    f_slot = 32 * KPB_g[g_of] + off_in_chunk

    # tile rank within (core, chunk, p0, dj)
    cellkey = ((dc * NCH + g_of) * P + sp0) * P + dj
    trank = _ranks_of(cellkey)

    T_g = [0] * NCH
    for g in range(NCH):
        T_g[g] = int(trank[g_of == g].max()) + 1

    # expansion: within (core, chunk, p0), overflow source-groups (any edge
    # with trank >= 15) go last, groups ordered by q0, runs by src
    ekey = (dc * NCH + g_of) * P + sp0
    ov_edge = trank >= TILES_PER_CALL
    ov_grp = np.zeros(NCORES * NCH * P * QW, bool)
    gq = ekey * QW + sq0
    ov_grp[gq[ov_edge]] = True
    is_ov = ov_grp[gq]
    subkey = is_ov * QW + sq0                           # overflow groups last
    m_pos, src_rank = _grp_pos(ekey, subkey)
    assert int(src_rank.max()) < RUN_CAP * SEED_REG, src_rank.max()

    NEK = NCORES * NCH * P
    cnt_tot = np.bincount(ekey, minlength=NEK)
    cnt_ov = np.bincount(ekey[is_ov], minlength=NEK)
    M1_g = [0] * NCH
    W2_g = [0] * NCH
    for g in range(NCH):
        sel = g_of == g
        M1_g[g] = _ceil(int(cnt_tot[ekey[sel]].max()), 2) * 2
        if T_g[g] > TILES_PER_CALL:
            W2_g[g] = max(2, _ceil(int(cnt_ov[ekey[sel]].max()), 2) * 2)
    # right-align each cell's overflow block at the end of the uniform
    # [0, M1_g) window (gap between the two blocks stays zero / w=0)
    M1_of_e = np.asarray(M1_g)[g_of]
    m_pos = np.where(is_ov,
                     M1_of_e - cnt_ov[ekey] + (m_pos - (cnt_tot[ekey]
                                                        - cnt_ov[ekey])),
                     m_pos)
    EB = np.r_[0, np.cumsum(M1_g)]
    MEXP = int(EB[-1])

    r1_calls = []                                       # (g, t0, t1, w)
    for g in range(NCH):
        t1 = min(TILES_PER_CALL, T_g[g])
        r1_calls.append((g, 0, t1, M1_g[g]))
        if T_g[g] > TILES_PER_CALL:
            r1_calls.append((g, TILES_PER_CALL, T_g[g], W2_g[g]))

    fill_rounds = [r for r in (1, 2, 4, 8) if r < RUN_CAP]

    # ---------- per-core tables ----------
    cores = []
    for c in range(NCORES):
        em = np.where(dc == c)[0]
        gg, j = g_of[em], dj[em]
        p0, q0, mp, sr, tr = sp0[em], sq0[em], m_pos[em], src_rank[em], trank[em]
        f = f_slot[em]
        w = w_all[em]
        m_glob = EB[gg] + mp

        seedidx = np.full((NCH, P, SEED_REG * QW), -1, np.int16)
        sm = sr % RUN_CAP == 0
        seedidx[gg[sm], p0[sm], (sr[sm] // RUN_CAP) * QW + q0[sm]] = \
            mp[sm].astype(np.int16)

        masks = np.zeros((len(fill_rounds), P, MEXP), np.float16)
        dist = sr % RUN_CAP
        for ri, rr in enumerate(fill_rounds):
            mm = dist >= rr
            masks[ri, p0[mm], m_glob[mm]] = 1.0

        w_exp = np.zeros((P, MEXP), np.float16)
        w_exp[p0, m_glob] = w.astype(np.float16)

        idx1 = []
        for (g2, t0, t1, wdt) in r1_calls:
            sel = (gg == g2) & (tr >= t0) & (tr < t1)
            idx = np.full((P, wdt), -1, np.int16)
            col = mp[sel] - (M1_g[g2] - wdt)
            assert col.min() >= 0 if sel.any() else True
            idx[p0[sel], col] = (128 * (tr[sel] - t0) + j[sel]).astype(np.int16)
            idx1.append(idx)

        idx2 = []
        for g2 in range(NCH):
            sel = gg == g2
            idx = np.full((P, 128 * T_g[g2]), -1, np.int16)
            idx[j[sel], 128 * tr[sel] + p0[sel]] = \
                (f[sel] - 32 * KPB_g[g2]).astype(np.int16)
            idx2.append(idx)

        b_in_t = np.zeros((P, Kreal), np.float32)
        nid = np.where(node_core == c)[0]
        b_in_t[node_row[nid], node_k[nid]] = b_in_full[nid]

        cores.append(dict(seedidx=seedidx, masks=masks, w_exp=w_exp,
                          idx1=idx1, idx2=idx2, b_in_t=b_in_t))

    meta = dict(Kreal=Kreal, K_g=K_g, NR2_g=NR2_g, KB_g=KB_g, KP_g=KP_g,
                KPB_g=KPB_g, KP=KP, T_g=T_g, M1_g=M1_g, W2_g=W2_g, EB=EB,
                MEXP=MEXP, r1_calls=r1_calls, fill_rounds=fill_rounds)
    return cores, perm, meta


def _act_np(v):
    y1 = np.maximum(v, np.float32(LEAK) * v)
    ysat = (1.0 - 0.25 / np.maximum(v, 0.5)).astype(v.dtype)
    return np.where(v > 0.5, ysat, y1)


def _sim(cores, perm, meta, n_iters, quant=True):
    """numpy mirror of the device program, for table validation."""
    dt = np.float16 if quant else np.float32
    Kreal, K_g, KB_g, KP_g, KPB_g, KP = (meta["Kreal"], meta["K_g"],
                                         meta["KB_g"], meta["KP_g"],
                                         meta["KPB_g"], meta["KP"])
    T_g, M1_g, EB, MEXP = meta["T_g"], meta["M1_g"], meta["EB"], meta["MEXP"]
    NR2_g = meta["NR2_g"]
    y = np.zeros(NC_PAD, np.float32)
    for it in range(n_iters):
        y2d = y.reshape(P, QW).astype(dt)
        y_next = np.zeros(NC_PAD, np.float32)
        for c, tb in enumerate(cores):
            exp_t = np.zeros((P, MEXP), dt)
            for g in range(NCH):
                sidx = tb["seedidx"][g]
                pp, cc = np.where(sidx >= 0)
                exp_t[pp, EB[g] + sidx[pp, cc]] = y2d[pp, cc % QW]
            for ri, rr in enumerate(meta["fill_rounds"]):
                sh = np.zeros_like(exp_t)
                sh[:, rr:] = exp_t[:, :-rr]
                exp_t = (exp_t + tb["masks"][ri].astype(dt) * sh).astype(dt)
            prod = (exp_t.astype(np.float32)
                    * tb["w_exp"].astype(np.float32)).astype(dt)
            vv = np.zeros((P, Kreal), np.float32)
            for g in range(NCH):
                stag = np.zeros((P, 128 * T_g[g]), dt)
                for ci, (g2, t0, t1, wdt) in enumerate(meta["r1_calls"]):
                    if g2 != g:
                        continue
                    idx = tb["idx1"][ci]
                    data = prod[:, EB[g] + M1_g[g] - wdt:EB[g] + M1_g[g]]
                    pp, cc = np.where(idx >= 0)
                    stag[pp, 128 * t0 + idx[pp, cc]] = data[pp, cc]
                t2 = np.zeros_like(stag)
                for t in range(T_g[g]):
                    t2[:, 128 * t:128 * (t + 1)] = stag[:, 128 * t:128 * (t + 1)].T
                slots = np.zeros((P, 32 * KP_g[g]), dt)
                idx = tb["idx2"][g]
                pp, cc = np.where(idx >= 0)
                slots[pp, idx[pp, cc]] = t2[pp, cc]
                n2 = NR2_g[g]
                s2 = slots[:, :64 * n2].reshape(P, n2, 64)
                s1 = slots[:, 64 * n2:].reshape(P, K_g[g] - n2, 32)
                red = np.concatenate(
                    [s2.astype(np.float32).sum(2),
                     s1.astype(np.float32).sum(2)], axis=1).astype(dt)
                vv[:, KB_g[g]:KB_g[g + 1]] = red.astype(np.float32)
            vv += tb["b_in_t"]
            y32 = _act_np(vv)
            jj, kk2 = np.meshgrid(np.arange(P), np.arange(Kreal), indexing="ij")
            y_next[SHARD * c + K16 * jj.ravel() + kk2.ravel()] = y32.ravel()
        y = y_next
    return y


# ============================ BASS KERNEL ============================

def _build(cores, meta, n_iters, no_cc=False):
    import concourse.bacc as bacc
    import concourse.bass as bass
    import concourse.mybir as mybir
    import concourse.tile as tile
    from concourse.masks import make_identity

    f16, f32, i16 = mybir.dt.float16, mybir.dt.float32, mybir.dt.int16
    AOP = mybir.AluOpType
    Kreal, K_g, KB_g, KP_g, KPB_g, KP = (meta["Kreal"], meta["K_g"],
                                         meta["KB_g"], meta["KP_g"],
                                         meta["KPB_g"], meta["KP"])
    T_g, M1_g, EB, MEXP = meta["T_g"], meta["M1_g"], meta["EB"], meta["MEXP"]
    NR2_g = meta["NR2_g"]
    r1_calls, fill_rounds = meta["r1_calls"], meta["fill_rounds"]
    NFILL = len(fill_rounds)

    nc = bacc.Bacc("TRN2", target_bir_lowering=False)

    d_seed = [nc.dram_tensor(f"t_seed{g}", [P, SEED_REG * QW], i16,
                             kind="ExternalInput")
              for g in range(NCH)]
    d_mask = [nc.dram_tensor(f"t_mask{r}", [P, MEXP], f16, kind="ExternalInput")
              for r in range(NFILL)]
    d_wexp = nc.dram_tensor("t_wexp", [P, MEXP], f16, kind="ExternalInput")
    d_idx1 = [nc.dram_tensor(f"t_idx1_{ci}", [P, wdt], i16,
                             kind="ExternalInput")
              for ci, (g2, _, _, wdt) in enumerate(r1_calls)]
    d_idx2 = [nc.dram_tensor(f"t_idx2_{g}", [P, 128 * T_g[g]], i16,
                             kind="ExternalInput") for g in range(NCH)]
    d_bin = nc.dram_tensor("t_bin", [P, Kreal], f32, kind="ExternalInput")
    d_yout = nc.dram_tensor("y_out", [P, Kreal], f32, kind="ExternalOutput")
    d_ysh = nc.dram_tensor("y_shard", [1, SHARD], f16, kind="Internal")
    d_yfull = nc.dram_tensor("y_full", [P, QW], f16, kind="Internal",
                             addr_space="Shared")

    r1_by_g = {}
    for ci, (g2, t0, t1, wdt) in enumerate(r1_calls):
        r1_by_g.setdefault(g2, []).append((ci, t0, t1, wdt))

    with tile.TileContext(nc) as tc:
        with tc.tile_pool(name="tables", bufs=1) as tp, \
             tc.tile_pool(name="psum", bufs=8, space="PSUM") as pp:
            t_seed = [tp.tile([P, SEED_REG * QW], i16, name=f"seed{g}")
                      for g in range(NCH)]
            t_mask = [tp.tile([P, MEXP], f16, name=f"mask{r}")
                      for r in range(NFILL)]
            t_wexp = tp.tile([P, MEXP], f16, name="wexp")
            t_idx1 = [tp.tile([P, wdt], i16, name=f"i1_{ci}")
                      for ci, (g2, _, _, wdt) in enumerate(r1_calls)]
            t_idx2 = [tp.tile([P, 128 * T_g[g]], i16, name=f"i2_{g}")
                      for g in range(NCH)]
            t_bin = tp.tile([P, Kreal], f32, name="bin")
            ident = tp.tile([P, P], f16, name="ident")
            y2d = tp.tile([P, SEED_REG * QW], f16, name="y2d")
            expb = [tp.tile([P, M1_g[g]], f16, name=f"expb{g}")
                    for g in range(NCH)]
            tmpb = tp.tile([P, max(M1_g)], f16, name="tmpb")
            stag = [tp.tile([P, 128 * T_g[g]], f16, name=f"stag{g}")
                    for g in range(NCH)]
            t2d = [tp.tile([P, 128 * T_g[g]], f16, name=f"t2d{g}")
                   for g in range(NCH)]
            slots = [tp.tile([P, 32 * KP_g[g]], f16, name=f"slots{g}")
                     for g in range(NCH)]
            sp16 = tp.tile([P, Kreal], f16, name="sp16")
            vv = tp.tile([P, Kreal], f32, name="vv")
            y1b = tp.tile([P, Kreal], f32, name="y1b")
            rb = tp.tile([P, Kreal], f32, name="rb")
            mb = tp.tile([P, Kreal], f32, name="mb")
            y32 = tp.tile([P, Kreal], f32, name="y32")
            y16 = tp.tile([P, Kreal], f16, name="y16")

            for g in range(NCH):
                nc.sync.dma_start(t_seed[g][:], d_seed[g][:])
                nc.sync.dma_start(t_idx2[g][:], d_idx2[g][:])
            for r in range(NFILL):
                nc.sync.dma_start(t_mask[r][:], d_mask[r][:])
            for ci in range(len(r1_calls)):
                nc.sync.dma_start(t_idx1[ci][:], d_idx1[ci][:])
            nc.sync.dma_start(t_wexp[:], d_wexp[:])
            nc.sync.dma_start(t_bin[:], d_bin[:])
            make_identity(nc, ident[:])
            nc.vector.memset(y2d[:], 0.0)
            nc.vector.memset(tmpb[:], 0.0)

            pool_chain = []

            def _chain(inst):
                if pool_chain:
                    bass._add_dep_helper(inst.ins, pool_chain[-1].ins,
                                         sync=True, reason="pool order")
                pool_chain.append(inst)

            dve_chain = []
            act_chain = []

            def _chain_on(lst, inst):
                if lst:
                    bass._add_dep_helper(inst.ins, lst[-1].ins,
                                         sync=True, reason="engine order")
                lst.append(inst)

            def _dve(inst):
                _chain_on(dve_chain, inst)

            def seed(g):
                _chain(nc.gpsimd.local_scatter(
                    expb[g][:], y2d[:], t_seed[g][:],
                    channels=P, num_elems=M1_g[g], num_idxs=SEED_REG * QW))

            def fill(g):
                w0, w1 = int(EB[g]), int(EB[g + 1])
                mw = M1_g[g]
                for ri, rr in enumerate(fill_rounds):
                    _dve(nc.vector.tensor_tensor(
                        tmpb[:, rr:mw], expb[g][:, 0:mw - rr],
                        t_mask[ri][:, w0 + rr:w1], op=AOP.mult))
                    _dve(nc.vector.tensor_tensor(
                        expb[g][:, rr:mw], expb[g][:, rr:mw],
                        tmpb[:, rr:mw], op=AOP.add))
                _dve(nc.vector.tensor_tensor(expb[g][:], expb[g][:],
                                             t_wexp[:, w0:w1], op=AOP.mult))

            def r1(g):
                for ci, t0, t1, wdt in r1_by_g[g]:
                    _chain(nc.gpsimd.local_scatter(
                        stag[g][:, 128 * t0:128 * t1],
                        expb[g][:, M1_g[g] - wdt:M1_g[g]],
                        t_idx1[ci][:], channels=P,
                        num_elems=128 * (t1 - t0), num_idxs=wdt))

            def transposes(g):
                for tb0 in range(0, T_g[g], 8):
                    nb = min(8, T_g[g] - tb0)
                    pt = pp.tile([P, 8 * P], f16, space="PSUM", tag="tr",
                                 name="tr")
                    for t in range(tb0, tb0 + nb):
                        nc.tensor.transpose(
                            pt[:, 128 * (t - tb0):128 * (t - tb0 + 1)],
                            stag[g][:, 128 * t:128 * (t + 1)], ident[:])
                    _chain_on(act_chain, nc.scalar.copy(
                        t2d[g][:, 128 * tb0:128 * (tb0 + nb)],
                        pt[:, 0:128 * nb]))

            def r2(g):
                _chain(nc.gpsimd.local_scatter(
                    slots[g][:], t2d[g][:],
                    t_idx2[g][:], channels=P, num_elems=32 * KP_g[g],
                    num_idxs=128 * T_g[g]))

            def reduce_g(g):
                n2 = NR2_g[g]
                kb = int(KB_g[g])
                # fp16 slot sums: |products| <= 0.2, segment <= 64 terms, and
                # the 2e-2 harness gate has ~30x margin over the fp16 error
                with nc.allow_low_precision(reason="fp16 slot sums, ample margin"):
                    if n2:
                        _dve(nc.vector.tensor_reduce(
                            sp16[:, kb:kb + n2],
                            slots[g][:, 0:64 * n2].rearrange(
                                "p (k s) -> p k s", s=64),
                            axis=mybir.AxisListType.X, op=AOP.add))
                    _dve(nc.vector.tensor_reduce(
                        sp16[:, kb + n2:kb + K_g[g]],
                        slots[g][:, 64 * n2:32 * KP_g[g]].rearrange(
                            "p (k s) -> p k s", s=32),
                        axis=mybir.AxisListType.X, op=AOP.add))

            def act_g(g):
                a, b2 = int(KB_g[g]), int(KB_g[g + 1])
                s = slice(a, b2)
                _dve(nc.vector.tensor_copy(vv[:, s], sp16[:, s]))
                _dve(nc.vector.tensor_tensor(vv[:, s], vv[:, s], t_bin[:, s],
                                             op=AOP.add))
                _dve(nc.vector.scalar_tensor_tensor(
                    y1b[:, s], vv[:, s], float(LEAK), vv[:, s], op0=AOP.mult,
                    op1=AOP.max))
                _dve(nc.vector.tensor_scalar_max(rb[:, s], vv[:, s], 0.5))
                _dve(nc.vector.reciprocal(rb[:, s], rb[:, s]))
                _dve(nc.vector.tensor_scalar(rb[:, s], rb[:, s], -0.25, 1.0,
                                             op0=AOP.mult, op1=AOP.add))
                _dve(nc.vector.tensor_scalar(mb[:, s], vv[:, s], 0.5, None,
                                             op0=AOP.is_gt))
                _dve(nc.vector.tensor_tensor(rb[:, s], rb[:, s], y1b[:, s],
                                             op=AOP.subtract))
                _dve(nc.vector.tensor_tensor(mb[:, s], mb[:, s], rb[:, s],
                                             op=AOP.mult))
                _dve(nc.vector.tensor_tensor(y32[:, s], y1b[:, s], mb[:, s],
                                             op=AOP.add))
                _dve(nc.vector.tensor_copy(y16[:, s], y32[:, s]))

            def exchange():
                nc.sync.dma_start(
                    d_ysh[:].rearrange("o (p k) -> (o p) k", p=P), y16[:])
                if not no_cc:
                    nc.gpsimd.collective_compute(
                        "AllGather", AOP.bypass,
                        replica_groups=[list(range(NCORES))],
                        ins=[d_ysh[:]], outs=[d_yfull[:]])
                for rgn in range(SEED_REG):
                    nc.sync.dma_start(y2d[:, rgn * QW:(rgn + 1) * QW],
                                      d_yfull[:])

            for _ in range(n_iters):
                seed(0); seed(1); seed(2)
                fill(0)
                r1(0)
                fill(1)
                r1(1)
                transposes(0)
                fill(2)
                r2(0)
                r1(2)
                transposes(1)
                reduce_g(0)
                act_g(0)
                r2(1)
                transposes(2)
                reduce_g(1)
                act_g(1)
                r2(2)
                reduce_g(2)
                act_g(2)
                exchange()
            nc.sync.dma_start(d_yout[:], y32[:])

    nc.compile()
    return nc


def _in_maps(cores, meta):
    maps = []
    for tb in cores:
        m = {"t_wexp": tb["w_exp"], "t_bin": tb["b_in_t"]}
        for g in range(NCH):
            m[f"t_seed{g}"] = tb["seedidx"][g]
            m[f"t_idx2_{g}"] = tb["idx2"][g]
        for r in range(len(meta["fill_rounds"])):
            m[f"t_mask{r}"] = np.ascontiguousarray(tb["masks"][r])
        for ci in range(len(meta["r1_calls"])):
            m[f"t_idx1_{ci}"] = tb["idx1"][ci]
        maps.append(m)
    return maps


def kernel(**inputs):
    from concourse.bass_utils import run_bass_kernel_spmd
    inputs = {k: np.asarray(v) for k, v in inputs.items()}
    cores, perm, meta = _prep(**inputs)
    nc = _build(cores, meta, ITERS)
    maps = _in_maps(cores, meta)
    res = run_bass_kernel_spmd(nc, [dict(m) for m in maps],
                               core_ids=list(range(NCORES)))
    Kreal = meta["Kreal"]
    y_full = np.zeros(NC_PAD, np.float32)
    jj, kk2 = np.meshgrid(np.arange(P), np.arange(Kreal), indexing="ij")
    for c in range(NCORES):
        y32 = res.results[c]["y_out"]
        y_full[SHARD * c + K16 * jj.ravel() + kk2.ravel()] = y32.ravel()
    y_old = y_full[perm]
    out = (inputs["out_weights"].astype(np.float32)
           * y_old[inputs["out_indices"]])[None, :]
    return out.astype(np.float32)


if __name__ == "__main__":
    import sys, time
    sys.path.insert(0, "/root/problem")
    import reference
    inputs = {k: np.asarray(v) for k, v in reference.setup_inputs().items()}
    t0 = time.time()
    cores, perm, meta = _prep(**inputs)
    print(f"prep {time.time()-t0:.1f}s K_g={meta['K_g']} NR2_g={meta['NR2_g']} "
          f"KP_g={meta['KP_g']} T_g={meta['T_g']} M1_g={meta['M1_g']} "
          f"W2_g={meta['W2_g']} MEXP={meta['MEXP']} "
          f"r1_calls={meta['r1_calls']}")
    if "sim" in sys.argv:
        n_it = int(sys.argv[sys.argv.index("sim") + 1])
        import jax.numpy as jnp
        ni = np.asarray(jnp.zeros((N,), jnp.float32).at[
            jnp.asarray(inputs["in_indices"])].set(
            jnp.asarray(inputs["in_weights"], jnp.float32)
            * jnp.asarray(inputs["x"][0], jnp.float32)))
        b_in = (ni + inputs["biases"]).astype(np.float64)
        rw = inputs["rec_weights"].astype(np.float64)
        er, ec = inputs["edge_rows"], inputs["edge_cols"]
        yref = np.zeros(N, np.float64)
        for _ in range(n_it):
            s = np.bincount(er, weights=rw * yref[ec], minlength=N)
            v = s + b_in
            yref = np.where(v > 0.5, 1.0 - 0.25 / np.maximum(v, 0.5),
                            np.maximum(v, LEAK * v))
        scale = np.abs(yref).max()
        t0 = time.time()
        ys = _sim(cores, perm, meta, n_it, quant=False)
        print(f"sim(noquant,{n_it}) {time.time()-t0:.1f}s  max rel err:",
              np.abs(ys[perm] - yref).max() / scale)
        t0 = time.time()
        ysq = _sim(cores, perm, meta, n_it, quant=True)
        print(f"sim(fp16,{n_it}) {time.time()-t0:.1f}s  max rel err:",
              np.abs(ysq[perm] - yref).max() / scale)


# revision 7
# speedup vs baseline: 1.3031x; 1.0140x over previous
"""Bionetwork sparse-matvec recurrence on 8 trn2 NeuronCores — v2.

y_{t+1} = act(A y_t + b_in), 150 iterations, A fixed sparse (3.2M edges,
100k nodes).  Dest-sharded across 8 cores; all routing tables SBUF-resident.

v2 changes vs v1:
  - chunks balanced by edge count AND tile-cell maxima (greedy + swaps),
    cutting GPSIMD staging scans
  - overflow tiles (trank >= 15) packed into an expansion suffix so the
    second round-1 scatter call scans a narrow table
  - class-2 destination columns get interleaved 64-wide slots (single
    segmented reduce, no fold passes)
  - pipeline-ordered emission so Pool/DVE/PE/Act overlap across chunks
"""
import numpy as np

N = 100000
E = 3200000
P = 128
NCORES = 8
K16 = 98                    # y16 columns per core
QW = NCORES * K16           # 784  canonical y2d width
NC_PAD = P * QW             # 100352
SHARD = P * K16             # 12544
NCH = 3
ITERS = 150
LEAK = 0.01
RUN_CAP = 8                 # fill rounds 1,2,4; seeds every 8
SEED_REG = 2                # seed regions (src_rank 0 and 8)
SW2 = 98                    # region-2 strip width (q0 < 98 <=> j % 8 == 0)
TILES_PER_CALL = 15         # 128*15 = 1920 <= 2046 local_scatter cap
MAX_SLOT_ELEMS = 2046


def _ceil(a, b):
    return -(-a // b)


def _ranks_of(key):
    """rank of each element within its equal-key group (stable)."""
    ne = key.shape[0]
    so = np.argsort(key, kind="stable")
    ks = key[so]
    st = np.r_[0, np.flatnonzero(np.diff(ks)) + 1]
    sid = np.zeros(ne, np.int64)
    sid[st[1:]] = 1
    sid = np.cumsum(sid)
    r = np.arange(ne) - st[sid]
    out = np.empty(ne, np.int64)
    out[so] = r
    return out


def _grp_pos(key, subsort):
    """m_pos: rank within key-group under (key, subsort) stable order, and
    src_rank: rank within (key, subsort)-group."""
    ne = key.shape[0]
    so = np.lexsort((subsort, key))
    kso = key[so]
    st = np.r_[0, np.flatnonzero(np.diff(kso)) + 1]
    sid = np.zeros(ne, np.int64)
    sid[st[1:]] = 1
    sid = np.cumsum(sid)
    mp = np.empty(ne, np.int64)
    mp[so] = np.arange(ne) - st[sid]
    k2 = kso * (subsort.max() + 1) + subsort[so]
    st2 = np.r_[0, np.flatnonzero(np.diff(k2)) + 1]
    sid2 = np.zeros(ne, np.int64)
    sid2[st2[1:]] = 1
    sid2 = np.cumsum(sid2)
    sr = np.empty(ne, np.int64)
    sr[so] = np.arange(ne) - st2[sid2]
    return mp, sr


def _prep(x, in_weights, rec_weights, biases, out_weights,
          in_indices, edge_rows, edge_cols, out_indices):
    deg = np.bincount(edge_rows, minlength=N)
    npseudo = np.maximum(1, _ceil(deg, 32))
    assert npseudo.max() <= 2, f"max in-degree {deg.max()} > 64 unsupported"

    # deal dests round-robin over 1024 (core,row) bins, heavy nodes first so
    # pseudo-slot classes are (nearly) a function of the column index
    rng = np.random.default_rng(12345)
    order = np.lexsort((rng.permutation(N), -npseudo))
    i = np.arange(N)
    b = i % (NCORES * P)
    node_core = np.empty(N, np.int64)
    node_row = np.empty(N, np.int64)
    node_kk = np.empty(N, np.int64)
    node_core[order] = b % NCORES
    node_row[order] = b // NCORES
    node_kk[order] = i // (NCORES * P)
    Kreal = int(node_kk.max()) + 1
    assert Kreal == K16

    col_np = np.zeros(Kreal, np.int64)
    np.maximum.at(col_np, node_kk, npseudo)

    # ---------- greedy column->chunk packing ----------
    flat0 = SHARD * node_core + K16 * node_row + node_kk
    dc = node_core[edge_rows]
    dj = node_row[edge_rows]
    dkk = node_kk[edge_rows]
    sp0 = flat0[edge_cols] // QW
    cellidx = ((dkk * NCORES + dc) * P + dj) * P + sp0
    colcells = np.bincount(cellidx, minlength=Kreal * NCORES * P * P)
    colcells = colcells.reshape(Kreal, NCORES, P, P).astype(np.int32)
    coledges = colcells.reshape(Kreal, -1).sum(axis=1)

    chunk_cells = np.zeros((NCH, NCORES, P, P), np.int32)
    chunk_edges = np.zeros(NCH, np.int64)
    chunk_cols = [[] for _ in range(NCH)]
    # small first chunk (short post-collective ramp), small-ish last (tail)
    SIZES = [33, 33, 32]
    CAP2 = [(MAX_SLOT_ELEMS // 32) - s for s in SIZES]   # class-2 col caps
    n2_cnt = [0] * NCH
    for k in np.argsort(-coledges):
        best, bkey = None, None
        for g in range(NCH):
            if len(chunk_cols[g]) >= SIZES[g]:
                continue
            if col_np[k] >= 2 and n2_cnt[g] >= CAP2[g]:
                continue
            cand = chunk_cells[g] + colcells[k]
            mx = int(cand.max())
            over = int((cand >= mx - 1).sum())
            key = (mx, over, chunk_edges[g])
            if bkey is None or key < bkey:
                best, bkey = g, key
        g = best
        chunk_cells[g] += colcells[k]
        chunk_edges[g] += coledges[k]
        chunk_cols[g].append(int(k))
        if col_np[k] >= 2:
            n2_cnt[g] += 1

    # local search: same-class column swaps minimizing sum of per-chunk tile
    # maxima (= Pool staging scans), tie-broken by the sorted maxima
    def _key(maxima):
        return (sum(maxima), sorted(maxima, reverse=True))
    cur = _key([int(chunk_cells[g].max()) for g in range(NCH)])
    rng2 = np.random.default_rng(999)
    stall = 0
    for it in range(3000):
        if stall > 60:
            break
        maxima = [int(chunk_cells[g].max()) for g in range(NCH)]
        gmax = int(np.argmax(maxima)) if it % 2 == 0 else int(rng2.integers(NCH))
        hot = np.unravel_index(np.argmax(chunk_cells[gmax]),
                               chunk_cells[gmax].shape)
        cols_h = chunk_cols[gmax]
        contrib = np.array([colcells[k][hot] for k in cols_h])
        ka = cols_h[int(rng2.choice(np.argsort(-contrib)[:3]))]
        improved = False
        for gb in rng2.permutation(NCH):
            if gb == gmax:
                continue
            for kb in rng2.permutation(chunk_cols[gb])[:20]:
                kb = int(kb)
                if col_np[ka] != col_np[kb]:
                    continue
                na = chunk_cells[gmax] - colcells[ka] + colcells[kb]
                nb = chunk_cells[gb] - colcells[kb] + colcells[ka]
                ms = [int(na.max()) if g == gmax else
                      int(nb.max()) if g == gb else
                      int(chunk_cells[g].max()) for g in range(NCH)]
                ns = _key(ms)
                if ns < cur:
                    chunk_cells[gmax] = na
                    chunk_cells[gb] = nb
                    chunk_cols[gmax][chunk_cols[gmax].index(ka)] = kb
                    chunk_cols[gb][chunk_cols[gb].index(kb)] = int(ka)
                    ed = coledges[ka] - coledges[kb]
                    chunk_edges[gmax] -= ed
                    chunk_edges[gb] += ed
                    cur = ns
                    improved = True
                    break
            if improved:
                break
        stall = 0 if improved else stall + 1

    # final column order: per chunk, class-2 first then class-1 (kk asc)
    kmap = np.empty(Kreal, np.int64)
    K_g, NR2_g = [], []
    pos = 0
    for g in range(NCH):
        cols = sorted(chunk_cols[g], key=lambda k: (-col_np[k], k))
        for k in cols:
            kmap[k] = pos
            pos += 1
        K_g.append(len(cols))
        NR2_g.append(int(sum(1 for k in cols if col_np[k] >= 2)))
    assert pos == Kreal
    node_k = kmap[node_kk]

    # chunk geometry: per chunk, slot layout [class2 cols: 64-wide][class1: 32]
    KB_g = np.r_[0, np.cumsum(K_g)]
    KP_g = [K_g[g] + NR2_g[g] for g in range(NCH)]       # 32-slot columns
    assert all(32 * kp <= MAX_SLOT_ELEMS for kp in KP_g), KP_g
    KPB_g = np.r_[0, np.cumsum(KP_g)]
    KP = int(KPB_g[-1])
    chunk_of_k = np.empty(Kreal, np.int64)
    cp_of_k = np.empty(Kreal, np.int64)
    for g in range(NCH):
        sel = np.arange(KB_g[g], KB_g[g + 1])
        chunk_of_k[sel] = g
        cp_of_k[sel] = sel - KB_g[g]

    # relocate long-run sources (any (core,chunk) out-run >= RUN_CAP+1) into
    # rows j % 8 == 0, where q0 = (98j+k) %% 784 < 98, so the second seed
    # region only needs a 98-column strip
    g_of_dest = chunk_of_k[node_k[edge_rows]]
    runkey = edge_cols * (NCORES * NCH) + dc * NCH + g_of_dest
    runcnt = np.bincount(runkey, minlength=N * NCORES * NCH)
    long_mask = (runcnt.reshape(N, NCORES * NCH) > RUN_CAP).any(axis=1)
    long_nodes = np.flatnonzero(long_mask)
    rng3 = np.random.default_rng(777)
    for s in long_nodes:
        if node_row[s] % 8 == 0:
            continue
        k = node_k[s]
        cand = np.flatnonzero((node_k == k) & (node_row % 8 == 0)
                              & ~long_mask)
        t = int(rng3.choice(cand))
        node_core[s], node_core[t] = node_core[t], node_core[s]
        node_row[s], node_row[t] = node_row[t], node_row[s]
    dc = node_core[edge_rows]
    dj = node_row[edge_rows]

    # final canonical map
    flat = SHARD * node_core + K16 * node_row + node_k
    perm = flat
    p0_of = flat // QW
    q0_of = flat % QW

    import jax.numpy as jnp
    node_in = np.asarray(
        jnp.zeros((N,), jnp.float32).at[jnp.asarray(in_indices)].set(
            jnp.asarray(in_weights, jnp.float32) * jnp.asarray(x[0], jnp.float32)))
    b_in_full = node_in + biases.astype(np.float32)

    # ---------- per-edge geometry ----------
    dk = node_k[edge_rows]
    g_of = chunk_of_k[dk]
    sp0 = p0_of[edge_cols]
    sq0 = q0_of[edge_cols]
    w_all = rec_weights.astype(np.float32)

    # slot position: class-2 col c -> 64 slots at 64*cp; class-1 col ->
    # 32 slots at 64*NR2 + 32*(cp - NR2)
    destid = dc * (P * K16) + dj * K16 + dk
    drank = _ranks_of(destid)
    nr2 = np.array([NR2_g[g] for g in range(NCH)])
    cp = cp_of_k[dk]
    is2 = cp < nr2[g_of]
    off_in_chunk = np.where(is2, 64 * cp + drank,
                            64 * nr2[g_of] + 32 * (cp - nr2[g_of]) + drank)
    assert int((drank >= np.where(is2, 64, 32)).sum()) == 0
    f_slot = 32 * KPB_g[g_of] + off_in_chunk

    # tile rank within (core, chunk, p0, dj)
    cellkey = ((dc * NCH + g_of) * P + sp0) * P + dj
    trank = _ranks_of(cellkey)

    T_g = [0] * NCH
    for g in range(NCH):
        T_g[g] = int(trank[g_of == g].max()) + 1

    # expansion: within (core, chunk, p0), overflow source-groups (any edge
    # with trank >= 15) go last, groups ordered by q0, runs by src
    ekey = (dc * NCH + g_of) * P + sp0
    ov_edge = trank >= TILES_PER_CALL
    ov_grp = np.zeros(NCORES * NCH * P * QW, bool)
    gq = ekey * QW + sq0
    ov_grp[gq[ov_edge]] = True
    is_ov = ov_grp[gq]
    subkey = is_ov * QW + sq0                           # overflow groups last
    m_pos, src_rank = _grp_pos(ekey, subkey)
    assert int(src_rank.max()) < RUN_CAP * SEED_REG, src_rank.max()

    NEK = NCORES * NCH * P
    cnt_tot = np.bincount(ekey, minlength=NEK)
    cnt_ov = np.bincount(ekey[is_ov], minlength=NEK)
    M1_g = [0] * NCH
    W2_g = [0] * NCH
    for g in range(NCH):
        sel = g_of == g
        M1_g[g] = _ceil(int(cnt_tot[ekey[sel]].max()), 2) * 2
        if T_g[g] > TILES_PER_CALL:
            W2_g[g] = max(2, _ceil(int(cnt_ov[ekey[sel]].max()), 2) * 2)
    # right-align each cell's overflow block at the end of the uniform
    # [0, M1_g) window (gap between the two blocks stays zero / w=0)
    M1_of_e = np.asarray(M1_g)[g_of]
    m_pos = np.where(is_ov,
                     M1_of_e - cnt_ov[ekey] + (m_pos - (cnt_tot[ekey]
                                                        - cnt_ov[ekey])),
                     m_pos)
    EB = np.r_[0, np.cumsum(M1_g)]
    MEXP = int(EB[-1])

    r1_calls = []                                       # (g, t0, t1, w)
    for g in range(NCH):
        t1 = min(TILES_PER_CALL, T_g[g])
        r1_calls.append((g, 0, t1, M1_g[g]))
        if T_g[g] > TILES_PER_CALL:
            r1_calls.append((g, TILES_PER_CALL, T_g[g], W2_g[g]))

    fill_rounds = [r for r in (1, 2, 4, 8) if r < RUN_CAP]

    # ---------- per-core tables ----------
    cores = []
    for c in range(NCORES):
        em = np.where(dc == c)[0]
        gg, j = g_of[em], dj[em]
        p0, q0, mp, sr, tr = sp0[em], sq0[em], m_pos[em], src_rank[em], trank[em]
        f = f_slot[em]
        w = w_all[em]
        m_glob = EB[gg] + mp

        seedidx = np.full((NCH, P, QW + SW2), -1, np.int16)
        sm = sr % RUN_CAP == 0
        reg = sr[sm] // RUN_CAP
        assert int((q0[sm][reg > 0] >= SW2).sum()) == 0, "long src not in strip"
        seedidx[gg[sm], p0[sm], reg * QW + q0[sm]] = mp[sm].astype(np.int16)

        masks = np.zeros((len(fill_rounds), P, MEXP), np.float16)
        dist = sr % RUN_CAP
        for ri, rr in enumerate(fill_rounds):
            mm = dist >= rr
            masks[ri, p0[mm], m_glob[mm]] = 1.0

        w_exp = np.zeros((P, MEXP), np.float16)
        w_exp[p0, m_glob] = w.astype(np.float16)

        idx1 = []
        for (g2, t0, t1, wdt) in r1_calls:
            sel = (gg == g2) & (tr >= t0) & (tr < t1)
            idx = np.full((P, wdt), -1, np.int16)
            col = mp[sel] - (M1_g[g2] - wdt)
            assert col.min() >= 0 if sel.any() else True
            idx[p0[sel], col] = (128 * (tr[sel] - t0) + j[sel]).astype(np.int16)
            idx1.append(idx)

        idx2 = []
        for g2 in range(NCH):
            sel = gg == g2
            idx = np.full((P, 128 * T_g[g2]), -1, np.int16)
            idx[j[sel], 128 * tr[sel] + p0[sel]] = \
                (f[sel] - 32 * KPB_g[g2]).astype(np.int16)
            idx2.append(idx)

        b_in_t = np.zeros((P, Kreal), np.float32)
        nid = np.where(node_core == c)[0]
        b_in_t[node_row[nid], node_k[nid]] = b_in_full[nid]

        cores.append(dict(seedidx=seedidx, masks=masks, w_exp=w_exp,
                          idx1=idx1, idx2=idx2, b_in_t=b_in_t))

    meta = dict(Kreal=Kreal, K_g=K_g, NR2_g=NR2_g, KB_g=KB_g, KP_g=KP_g,
                KPB_g=KPB_g, KP=KP, T_g=T_g, M1_g=M1_g, W2_g=W2_g, EB=EB,
                MEXP=MEXP, r1_calls=r1_calls,
                fill_rounds=fill_rounds)
    return cores, perm, meta


def _act_np(v):
    y1 = np.maximum(v, np.float32(LEAK) * v)
    ysat = (1.0 - 0.25 / np.maximum(v, 0.5)).astype(v.dtype)
    return np.where(v > 0.5, ysat, y1)


def _sim(cores, perm, meta, n_iters, quant=True):
    """numpy mirror of the device program, for table validation."""
    dt = np.float16 if quant else np.float32
    Kreal, K_g, KB_g, KP_g, KPB_g, KP = (meta["Kreal"], meta["K_g"],
                                         meta["KB_g"], meta["KP_g"],
                                         meta["KPB_g"], meta["KP"])
    T_g, M1_g, EB, MEXP = meta["T_g"], meta["M1_g"], meta["EB"], meta["MEXP"]
    NR2_g = meta["NR2_g"]
    y = np.zeros(NC_PAD, np.float32)
    for it in range(n_iters):
        y2d = y.reshape(P, QW).astype(dt)
        y_next = np.zeros(NC_PAD, np.float32)
        for c, tb in enumerate(cores):
            exp_t = np.zeros((P, MEXP), dt)
            for g in range(NCH):
                sidx = tb["seedidx"][g]
                pp, cc = np.where(sidx >= 0)
                exp_t[pp, EB[g] + sidx[pp, cc]] = y2d[pp, cc % QW]
            for ri, rr in enumerate(meta["fill_rounds"]):
                sh = np.zeros_like(exp_t)
                sh[:, rr:] = exp_t[:, :-rr]
                exp_t = (exp_t + tb["masks"][ri].astype(dt) * sh).astype(dt)
            prod = (exp_t.astype(np.float32)
                    * tb["w_exp"].astype(np.float32)).astype(dt)
            vv = np.zeros((P, Kreal), np.float32)
            for g in range(NCH):
                stag = np.zeros((P, 128 * T_g[g]), dt)
                for ci, (g2, t0, t1, wdt) in enumerate(meta["r1_calls"]):
                    if g2 != g:
                        continue
                    idx = tb["idx1"][ci]
                    data = prod[:, EB[g] + M1_g[g] - wdt:EB[g] + M1_g[g]]
                    pp, cc = np.where(idx >= 0)
                    stag[pp, 128 * t0 + idx[pp, cc]] = data[pp, cc]
                t2 = np.zeros_like(stag)
                for t in range(T_g[g]):
                    t2[:, 128 * t:128 * (t + 1)] = stag[:, 128 * t:128 * (t + 1)].T
                slots = np.zeros((P, 32 * KP_g[g]), dt)
                idx = tb["idx2"][g]
                pp, cc = np.where(idx >= 0)
                slots[pp, idx[pp, cc]] = t2[pp, cc]
                n2 = NR2_g[g]
                s2 = slots[:, :64 * n2].reshape(P, n2, 64)
                s1 = slots[:, 64 * n2:].reshape(P, K_g[g] - n2, 32)
                red = np.concatenate(
                    [s2.astype(np.float32).sum(2),
                     s1.astype(np.float32).sum(2)], axis=1).astype(dt)
                vv[:, KB_g[g]:KB_g[g + 1]] = red.astype(np.float32)
            vv += tb["b_in_t"]
            y32 = _act_np(vv)
            jj, kk2 = np.meshgrid(np.arange(P), np.arange(Kreal), indexing="ij")
            y_next[SHARD * c + K16 * jj.ravel() + kk2.ravel()] = y32.ravel()
        y = y_next
    return y


# ============================ BASS KERNEL ============================

def _build(cores, meta, n_iters, no_cc=False):
    import concourse.bacc as bacc
    import concourse.bass as bass
    import concourse.mybir as mybir
    import concourse.tile as tile
    from concourse.masks import make_identity

    f16, f32, i16 = mybir.dt.float16, mybir.dt.float32, mybir.dt.int16
    AOP = mybir.AluOpType
    Kreal, K_g, KB_g, KP_g, KPB_g, KP = (meta["Kreal"], meta["K_g"],
                                         meta["KB_g"], meta["KP_g"],
                                         meta["KPB_g"], meta["KP"])
    T_g, M1_g, EB, MEXP = meta["T_g"], meta["M1_g"], meta["EB"], meta["MEXP"]
    NR2_g = meta["NR2_g"]
    r1_calls, fill_rounds = meta["r1_calls"], meta["fill_rounds"]
    NFILL = len(fill_rounds)

    nc = bacc.Bacc("TRN2", target_bir_lowering=False)

    d_seed = [nc.dram_tensor(f"t_seed{g}", [P, QW + SW2], i16,
                             kind="ExternalInput")
              for g in range(NCH)]
    d_mask = [nc.dram_tensor(f"t_mask{r}", [P, MEXP], f16, kind="ExternalInput")
              for r in range(NFILL)]
    d_wexp = nc.dram_tensor("t_wexp", [P, MEXP], f16, kind="ExternalInput")
    d_idx1 = [nc.dram_tensor(f"t_idx1_{ci}", [P, wdt], i16,
                             kind="ExternalInput")
              for ci, (g2, _, _, wdt) in enumerate(r1_calls)]
    d_idx2 = [nc.dram_tensor(f"t_idx2_{g}", [P, 128 * T_g[g]], i16,
                             kind="ExternalInput") for g in range(NCH)]
    d_bin = nc.dram_tensor("t_bin", [P, Kreal], f32, kind="ExternalInput")
    d_yout = nc.dram_tensor("y_out", [P, Kreal], f32, kind="ExternalOutput")
    d_ysh = nc.dram_tensor("y_shard", [1, SHARD], f16, kind="Internal")
    d_yfull = nc.dram_tensor("y_full", [P, QW], f16, kind="Internal",
                             addr_space="Shared")

    r1_by_g = {}
    for ci, (g2, t0, t1, wdt) in enumerate(r1_calls):
        r1_by_g.setdefault(g2, []).append((ci, t0, t1, wdt))

    with tile.TileContext(nc) as tc:
        with tc.tile_pool(name="tables", bufs=1) as tp, \
             tc.tile_pool(name="psum", bufs=8, space="PSUM") as pp:
            t_seed = [tp.tile([P, QW + SW2], i16, name=f"seed{g}")
                      for g in range(NCH)]
            t_mask = [tp.tile([P, MEXP], f16, name=f"mask{r}")
                      for r in range(NFILL)]
            t_wexp = tp.tile([P, MEXP], f16, name="wexp")
            t_idx1 = [tp.tile([P, wdt], i16, name=f"i1_{ci}")
                      for ci, (g2, _, _, wdt) in enumerate(r1_calls)]
            t_idx2 = [tp.tile([P, 128 * T_g[g]], i16, name=f"i2_{g}")
                      for g in range(NCH)]
            t_bin = tp.tile([P, Kreal], f32, name="bin")
            ident = tp.tile([P, P], f16, name="ident")
            y2d = tp.tile([P, QW + SW2], f16, name="y2d")
            expall = tp.tile([P, MEXP], f16, name="expall")
            expb = [expall[:, int(EB[g]):int(EB[g + 1])] for g in range(NCH)]
            tmpb = tp.tile([P, max(M1_g)], f16, name="tmpb")
            stag = [tp.tile([P, 128 * T_g[g]], f16, name=f"stag{g}")
                    for g in range(NCH)]
            t2d = [tp.tile([P, 128 * T_g[g]], f16, name=f"t2d{g}")
                   for g in range(NCH)]
            slots = [tp.tile([P, 32 * KP_g[g]], f16, name=f"slots{g}")
                     for g in range(NCH)]
            sp16 = tp.tile([P, Kreal], f16, name="sp16")
            vv = tp.tile([P, Kreal], f32, name="vv")
            y1b = tp.tile([P, Kreal], f32, name="y1b")
            rb = tp.tile([P, Kreal], f32, name="rb")
            mb = tp.tile([P, Kreal], f32, name="mb")
            y32 = tp.tile([P, Kreal], f32, name="y32")
            y16 = tp.tile([P, Kreal], f16, name="y16")

            for g in range(NCH):
                nc.sync.dma_start(t_seed[g][:], d_seed[g][:])
                nc.sync.dma_start(t_idx2[g][:], d_idx2[g][:])
            for r in range(NFILL):
                nc.sync.dma_start(t_mask[r][:], d_mask[r][:])
            for ci in range(len(r1_calls)):
                nc.sync.dma_start(t_idx1[ci][:], d_idx1[ci][:])
            nc.sync.dma_start(t_wexp[:], d_wexp[:])
            nc.sync.dma_start(t_bin[:], d_bin[:])
            make_identity(nc, ident[:])
            nc.vector.memset(y2d[:], 0.0)
            nc.vector.memset(tmpb[:], 0.0)

            pool_chain = []

            def _chain(inst):
                if pool_chain:
                    bass._add_dep_helper(inst.ins, pool_chain[-1].ins,
                                         sync=True, reason="pool order")
                pool_chain.append(inst)

            dve_chain = []
            act_chain = []

            def _chain_on(lst, inst):
                if lst:
                    bass._add_dep_helper(inst.ins, lst[-1].ins,
                                         sync=True, reason="engine order")
                lst.append(inst)

            def _dve(inst):
                _chain_on(dve_chain, inst)

            def seed(g):
                _chain(nc.gpsimd.local_scatter(
                    expall[:, int(EB[g]):int(EB[g + 1])], y2d[:], t_seed[g][:],
                    channels=P, num_elems=M1_g[g], num_idxs=QW + SW2))

            def fill(g):
                w0, w1 = int(EB[g]), int(EB[g + 1])
                mw = M1_g[g]
                eb = expall[:, w0:w1]
                for ri, rr in enumerate(fill_rounds):
                    _dve(nc.vector.tensor_tensor(
                        tmpb[:, rr:mw], eb[:, 0:mw - rr],
                        t_mask[ri][:, w0 + rr:w1], op=AOP.mult))
                    _dve(nc.vector.tensor_tensor(
                        eb[:, rr:mw], eb[:, rr:mw],
                        tmpb[:, rr:mw], op=AOP.add))
                _dve(nc.vector.tensor_tensor(eb, eb,
                                             t_wexp[:, w0:w1], op=AOP.mult))

            def r1(g):
                w1 = int(EB[g + 1])
                for ci, t0, t1, wdt in r1_by_g[g]:
                    _chain(nc.gpsimd.local_scatter(
                        stag[g][:, 128 * t0:128 * t1],
                        expall[:, w1 - wdt:w1],
                        t_idx1[ci][:], channels=P,
                        num_elems=128 * (t1 - t0), num_idxs=wdt))

            def transposes(g):
                for tb0 in range(0, T_g[g], 8):
                    nb = min(8, T_g[g] - tb0)
                    pt = pp.tile([P, 8 * P], f16, space="PSUM", tag="tr",
                                 name="tr")
                    for t in range(tb0, tb0 + nb):
                        nc.tensor.transpose(
                            pt[:, 128 * (t - tb0):128 * (t - tb0 + 1)],
                            stag[g][:, 128 * t:128 * (t + 1)], ident[:])
                    _chain_on(act_chain, nc.scalar.copy(
                        t2d[g][:, 128 * tb0:128 * (tb0 + nb)],
                        pt[:, 0:128 * nb]))

            def r2(g):
                _chain(nc.gpsimd.local_scatter(
                    slots[g][:], t2d[g][:],
                    t_idx2[g][:], channels=P, num_elems=32 * KP_g[g],
                    num_idxs=128 * T_g[g]))

            def reduce_g(g):
                n2 = NR2_g[g]
                kb = int(KB_g[g])
                # fp16 slot sums: |products| <= 0.2, segment <= 64 terms, and
                # the 2e-2 harness gate has ~30x margin over the fp16 error
                with nc.allow_low_precision(reason="fp16 slot sums, ample margin"):
                    if n2:
                        _dve(nc.vector.tensor_reduce(
                            sp16[:, kb:kb + n2],
                            slots[g][:, 0:64 * n2].rearrange(
                                "p (k s) -> p k s", s=64),
                            axis=mybir.AxisListType.X, op=AOP.add))
                    _dve(nc.vector.tensor_reduce(
                        sp16[:, kb + n2:kb + K_g[g]],
                        slots[g][:, 64 * n2:32 * KP_g[g]].rearrange(
                            "p (k s) -> p k s", s=32),
                        axis=mybir.AxisListType.X, op=AOP.add))

            def act_g(g):
                a, b2 = int(KB_g[g]), int(KB_g[g + 1])
                s = slice(a, b2)
                _dve(nc.vector.tensor_copy(vv[:, s], sp16[:, s]))
                _dve(nc.vector.tensor_tensor(vv[:, s], vv[:, s], t_bin[:, s],
                                             op=AOP.add))
                _dve(nc.vector.scalar_tensor_tensor(
                    y1b[:, s], vv[:, s], float(LEAK), vv[:, s], op0=AOP.mult,
                    op1=AOP.max))
                _dve(nc.vector.tensor_scalar_max(rb[:, s], vv[:, s], 0.5))
                _dve(nc.vector.reciprocal(rb[:, s], rb[:, s]))
                _dve(nc.vector.tensor_scalar(rb[:, s], rb[:, s], -0.25, 1.0,
                                             op0=AOP.mult, op1=AOP.add))
                _dve(nc.vector.tensor_scalar(mb[:, s], vv[:, s], 0.5, None,
                                             op0=AOP.is_gt))
                _dve(nc.vector.tensor_tensor(rb[:, s], rb[:, s], y1b[:, s],
                                             op=AOP.subtract))
                _dve(nc.vector.tensor_tensor(mb[:, s], mb[:, s], rb[:, s],
                                             op=AOP.mult))
                _dve(nc.vector.tensor_tensor(y32[:, s], y1b[:, s], mb[:, s],
                                             op=AOP.add))
                _dve(nc.vector.tensor_copy(y16[:, s], y32[:, s]))
                nc.sync.dma_start(
                    d_ysh[:].rearrange("o (p k) -> (o p) k", p=P)[:, s],
                    y16[:, s])

            def exchange():
                if not no_cc:
                    nc.gpsimd.collective_compute(
                        "AllGather", AOP.bypass,
                        replica_groups=[list(range(NCORES))],
                        ins=[d_ysh[:]], outs=[d_yfull[:]])
                nc.sync.dma_start(y2d[:, 0:QW], d_yfull[:])
                nc.sync.dma_start(y2d[:, QW:QW + SW2], d_yfull[:, 0:SW2])

            for _ in range(n_iters):
                seed(0); seed(1)
                fill(0)
                r1(0)
                seed(2)
                fill(1)
                r1(1)
                transposes(0)
                fill(2)
                r2(0)
                r1(2)
                transposes(1)
                reduce_g(0)
                act_g(0)
                r2(1)
                transposes(2)
                reduce_g(1)
                act_g(1)
                r2(2)
                reduce_g(2)
                act_g(2)
                exchange()
            nc.sync.dma_start(d_yout[:], y32[:])

    nc.compile()
    return nc


def _in_maps(cores, meta):
    maps = []
    for tb in cores:
        m = {"t_wexp": tb["w_exp"], "t_bin": tb["b_in_t"]}
        for g in range(NCH):
            m[f"t_seed{g}"] = tb["seedidx"][g]
            m[f"t_idx2_{g}"] = tb["idx2"][g]
        for r in range(len(meta["fill_rounds"])):
            m[f"t_mask{r}"] = np.ascontiguousarray(tb["masks"][r])
        for ci in range(len(meta["r1_calls"])):
            m[f"t_idx1_{ci}"] = tb["idx1"][ci]
        maps.append(m)
    return maps


def kernel(**inputs):
    from concourse.bass_utils import run_bass_kernel_spmd
    inputs = {k: np.asarray(v) for k, v in inputs.items()}
    cores, perm, meta = _prep(**inputs)
    nc = _build(cores, meta, ITERS)
    maps = _in_maps(cores, meta)
    res = run_bass_kernel_spmd(nc, [dict(m) for m in maps],
                               core_ids=list(range(NCORES)))
    Kreal = meta["Kreal"]
    y_full = np.zeros(NC_PAD, np.float32)
    jj, kk2 = np.meshgrid(np.arange(P), np.arange(Kreal), indexing="ij")
    for c in range(NCORES):
        y32 = res.results[c]["y_out"]
        y_full[SHARD * c + K16 * jj.ravel() + kk2.ravel()] = y32.ravel()
    y_old = y_full[perm]
    out = (inputs["out_weights"].astype(np.float32)
           * y_old[inputs["out_indices"]])[None, :]
    return out.astype(np.float32)


if __name__ == "__main__":
    import sys, time
    sys.path.insert(0, "/root/problem")
    import reference
    inputs = {k: np.asarray(v) for k, v in reference.setup_inputs().items()}
    t0 = time.time()
    cores, perm, meta = _prep(**inputs)
    print(f"prep {time.time()-t0:.1f}s K_g={meta['K_g']} NR2_g={meta['NR2_g']} "
          f"KP_g={meta['KP_g']} T_g={meta['T_g']} M1_g={meta['M1_g']} "
          f"W2_g={meta['W2_g']} MEXP={meta['MEXP']} "
          f"r1_calls={meta['r1_calls']}")
    if "sim" in sys.argv:
        n_it = int(sys.argv[sys.argv.index("sim") + 1])
        import jax.numpy as jnp
        ni = np.asarray(jnp.zeros((N,), jnp.float32).at[
            jnp.asarray(inputs["in_indices"])].set(
            jnp.asarray(inputs["in_weights"], jnp.float32)
            * jnp.asarray(inputs["x"][0], jnp.float32)))
        b_in = (ni + inputs["biases"]).astype(np.float64)
        rw = inputs["rec_weights"].astype(np.float64)
        er, ec = inputs["edge_rows"], inputs["edge_cols"]
        yref = np.zeros(N, np.float64)
        for _ in range(n_it):
            s = np.bincount(er, weights=rw * yref[ec], minlength=N)
            v = s + b_in
            yref = np.where(v > 0.5, 1.0 - 0.25 / np.maximum(v, 0.5),
                            np.maximum(v, LEAK * v))
        scale = np.abs(yref).max()
        t0 = time.time()
        ys = _sim(cores, perm, meta, n_it, quant=False)
        print(f"sim(noquant,{n_it}) {time.time()-t0:.1f}s  max rel err:",
              np.abs(ys[perm] - yref).max() / scale)
        t0 = time.time()
        ysq = _sim(cores, perm, meta, n_it, quant=True)
        print(f"sim(fp16,{n_it}) {time.time()-t0:.1f}s  max rel err:",
              np.abs(ysq[perm] - yref).max() / scale)
